# revision 11
# baseline (speedup 1.0000x reference)
"""GAT layer Bass kernel for Trainium2, 8-core SPMD.

Sharding: core c handles batch b = c//2 and row-half ih = c%2 (512 rows of i).

v1 design (vs v0 baseline):
- e-slabs shipped as bf16 in octet-contiguous layout (2KB descriptors,
  half the HBM bytes, no on-chip cast).
- The att1/att2/adj broadcast matmuls are replaced by DVE writes into
  PSUM: a per-block fused (att1 + att2 + cst) bf16 tile plus a per-octet
  scalar_tensor_tensor that adds the adjacency bias and seeds PSUM; the
  PE then only runs the 8 att_e matmuls per octet (block-diag ae_w trick)
  and attention@V.
- adj is shipped as bf16 bias slabs ((adj-1)*1e9: exactly 0 or -9.98e8).
"""
import sys
sys.path.insert(0, "/opt/trn_rl_repo")
from contextlib import ExitStack

import numpy as np

import concourse.bass as bass
import concourse.tile as tile
from concourse import mybir
from concourse.masks import make_identity

F32 = mybir.dt.float32
BF16 = mybir.dt.bfloat16
AF = mybir.ActivationFunctionType
OP = mybir.AluOpType

B, N = 4, 1024
FN, FH, FE, FG = 128, 128, 16, 128
OUT, H = 128, 8
DH = OUT // H          # 16
ZIN = FN + FH          # 256
NC = 8                 # cores
NI = N // 2            # own rows per core = 512
NJH, NJL = N // 8, 8   # j = j_hi*8 + j_lo
NBLK = NI // 128       # i-blocks per core = 4
NOCT = 128 // 8        # octets per block = 16


def build_core_program(nc, n_iters=1):
    d = {}
    def inp(name, shape, dtype=F32):
        d[name] = nc.dram_tensor(name, shape, dtype, kind="ExternalInput").ap()
    inp("e_oct", [NI // 8, 128, 1024], BF16)   # [oct, (j_lo,e), (i8, j_hi)]
    inp("adj_oct", [NOCT, 128, 4, 64], BF16)   # [grp, j_hi, oct4, (i8, j_lo8)]
    inp("nf", [N, FN])
    inp("hd", [N, FH])
    inp("gf", [FG, 1])
    inp("m_w", [ZIN, OUT]); inp("m_b", [1, OUT])
    inp("skip_w", [ZIN, OUT]); inp("skip_b", [1, OUT])
    inp("a1_w", [ZIN, H]); inp("a1_b", [1, H])
    inp("a2_w", [ZIN, H]); inp("a2_b", [1, H])
    inp("ae_w", [FE, H]); inp("ae_b", [1, H])
    inp("ag_w", [FG, H]); inp("ag_b", [1, H])
    ret = nc.dram_tensor("ret", [NI, OUT], F32, kind="ExternalOutput").ap()

    with tile.TileContext(nc) as tc:
        with ExitStack() as ctx:
            emit(ctx, tc, d, ret, n_iters)


def emit(ctx, tc, d, ret, n_iters):
    nc = tc.nc
    P = lambda name, bufs=1: ctx.enter_context(tc.tile_pool(name=name, bufs=bufs))
    PS = lambda name, bufs=1: ctx.enter_context(
        tc.tile_pool(name=name, bufs=bufs, space="PSUM"))

    const = P("const")          # long-lived constants / staging
    psc_pool = PS("ps_small", bufs=2)   # all [128, <=128] psum tiles, shared slots
    psmisc = PS("ps_misc", bufs=1)      # odd-shaped psum tiles
    lp_pool = PS("logits", bufs=4)      # [128, 512] f32 logit tiles
    class _PS:
        def tile(self, shape, dtype):
            return psc_pool.tile(shape, dtype, tag="s", name="pstile")
    psc = _PS()
    # ---------------- prologue: constants ----------------
    ident = const.tile([128, 128], F32)
    make_identity(nc, ident[:])
    ones_bf = const.tile([128, 128], BF16)
    nc.gpsimd.memset(ones_bf[:], 1.0)
    ones_row = const.tile([1, 128], F32)
    nc.gpsimd.memset(ones_row[:], 1.0)

    # small weights into sbuf; ZIN-row weights stored as [128, (2, n)]
    wpool = P("weights")
    def load(name, shape, dtype=F32):
        t = wpool.tile(shape, dtype, name=name)
        nc.gpsimd.dma_start(t[:], d[name][:])
        return t
    def load2(name, ncols):
        t = wpool.tile([128, 2, ncols], F32, name=name)
        nc.gpsimd.dma_start(t[:], d[name][:].rearrange("(c p) n -> p c n", c=2))
        return lambda ct: t[:, ct, :]
    m_w = load2("m_w", OUT);  m_b = load("m_b", [1, OUT])
    sk_w = load2("skip_w", OUT); sk_b = load("skip_b", [1, OUT])
    a1_w = load2("a1_w", H); a1_b = load("a1_b", [1, H])
    a2_w = load2("a2_w", H); a2_b = load("a2_b", [1, H])
    ae_w = load("ae_w", [FE, H]); ae_b = load("ae_b", [1, H])
    ag_w = load("ag_w", [FG, H]); ag_b = load("ag_b", [1, H])
    gf = load("gf", [FG, 1])

    # blockdiag bd[(j_lo,e), (j_lo', h)] = ae_w[e,h] * (j_lo == j_lo')
    bd = const.tile([128, 64], BF16)
    nc.gpsimd.memset(bd[:], 0.0)
    ae_w_bf = wpool.tile([FE, H], BF16, name="ae_w_bf")
    nc.vector.tensor_copy(ae_w_bf[:], ae_w[:])
    for jl in range(NJL):
        dst = bd[:].rearrange("p (j h) -> p j h", j=NJL)[jl * 16:(jl + 1) * 16, jl, :]
        nc.gpsimd.dma_start(dst, ae_w_bf[:, :])

    # zT: [c, j] two c-tiles of [128, 1024] f32
    zT = const.tile([128, 2 * N], F32)  # cols [0:1024] = nf.T, [1024:2048] = hd.T
    zpool = P("zstage", bufs=3)
    for half, src in ((0, d["nf"]), (1, d["hd"])):
        for jb in range(N // 128):
            st = zpool.tile([128, 128], F32)
            nc.gpsimd.dma_start(st[:], src[jb * 128:(jb + 1) * 128, :])
            tp = psc.tile([128, 128], F32)
            nc.tensor.transpose(tp[:], st[:], ident[:])
            nc.vector.tensor_copy(
                zT[:, half * N + jb * 128: half * N + (jb + 1) * 128], tp[:])

    def zT_half(h_idx):
        return zT[:, h_idx * N:(h_idx + 1) * N]

    # cst[h] = a1_b + a2_b + ae_b + ag_b + gf @ ag_w   (shape [1, 8])
    attg_ps = psmisc.tile([1, H], F32, tag="m", name="attg_ps")
    nc.tensor.matmul(attg_ps[:], gf[:], ag_w[:], start=True, stop=True)
    cstv = const.tile([1, H], F32)
    nc.vector.scalar_tensor_tensor(cstv[:], a1_b[:], 1.0, a2_b[:], OP.mult, OP.add)
    nc.vector.scalar_tensor_tensor(cstv[:], cstv[:], 1.0, ae_b[:], OP.mult, OP.add)
    nc.vector.scalar_tensor_tensor(cstv[:], cstv[:], 1.0, ag_b[:], OP.mult, OP.add)
    nc.vector.scalar_tensor_tensor(cstv[:], cstv[:], 1.0, attg_ps[:], OP.mult, OP.add)
    # broadcast cst to all 128 partitions: ones_row.T @ cstv
    cstb_ps = psmisc.tile([128, H], F32, tag="m", name="cstb_ps")
    nc.tensor.matmul(cstb_ps[:], ones_row[:], cstv[:], start=True, stop=True)
    cstb = const.tile([128, H], F32)
    nc.vector.tensor_copy(cstb[:], cstb_ps[:])

    # att2g[j_hi, (h, j_lo)] = att_2[j, h] + cst[h]   (f32 [128, 64])
    att2g = const.tile([128, 64], F32)
    for jl in range(NJL):
        a2ps = psc.tile([128, H], F32)
        for ct in range(2):
            lhs = zT_half(ct)[:].rearrange("p (j l) -> p j l", l=8)[:, :, jl]
            nc.tensor.matmul(a2ps[:], lhs, a2_w(ct),
                             start=(ct == 0), stop=(ct == 1))
        dst = att2g[:].rearrange("p (h j) -> p h j", h=H)[:, :, jl]
        nc.vector.scalar_tensor_tensor(dst, a2ps[:], 1.0, cstb[:], OP.mult, OP.add)

    # q_sum[k, (i, h)] bf16: folded z (x) a1_w product so that
    # ones.T @ q_sum = att_1[i, h] broadcast over all partitions.
    q_sum = const.tile([128, NI * H], BF16)
    qtmp = const.tile([128, NI * H], F32)
    for ct in range(2):
        zslice = zT_half(ct)[:, OWN_I0:OWN_I0 + NI]
        z3 = zslice.rearrange("p (i x) -> p i x", x=1).broadcast_to([128, NI, H])
        a3 = a1_w(ct).rearrange("p (x h) -> p x h", x=1).broadcast_to([128, NI, H])
        if ct == 0:
            nc.vector.scalar_tensor_tensor(
                qtmp[:].rearrange("p (i h) -> p i h", h=H), z3, 1.0, a3,
                OP.mult, OP.mult)
        else:
            q2 = const.tile([128, NI * H], F32)
            nc.vector.scalar_tensor_tensor(
                q2[:].rearrange("p (i h) -> p i h", h=H), z3, 1.0, a3,
                OP.mult, OP.mult)
            nc.vector.scalar_tensor_tensor(
                q_sum[:].rearrange("p (i h) -> p i h", h=H),
                qtmp[:].rearrange("p (i h) -> p i h", h=H), 1.0,
                q2[:].rearrange("p (i h) -> p i h", h=H), OP.mult, OP.add)

    # att1bc[p, (i, h)] bf16 [128, 4096]: att_1 broadcast over partitions
    att1bc = const.tile([128, NI * H], BF16)
    for hb in range(NI * H // 512):
        bps = lp_pool.tile([128, 512], F32, tag="L")
        nc.tensor.matmul(bps[:], ones_bf[:],
                         q_sum[:, hb * 512:(hb + 1) * 512],
                         start=True, stop=True)
        nc.scalar.copy(att1bc[:, hb * 512:(hb + 1) * 512], bps[:])

    # V_perm[j_hi, (h, j_lo, 17)] bf16; col 16 of each (h,j_lo) group is 1.0
    v_perm = const.tile([128, H * NJL * (DH + 1)], BF16)
    nc.gpsimd.memset(v_perm[:], 1.0)
    m_b_bc_ps = psc.tile([128, OUT], F32)
    nc.tensor.matmul(m_b_bc_ps[:], ones_row[:], m_b[:], start=True, stop=True)
    m_b_bc = const.tile([128, OUT], F32)
    nc.vector.tensor_copy(m_b_bc[:], m_b_bc_ps[:])
    for jl in range(NJL):
        vps = psc.tile([128, OUT], F32)
        for ct in range(2):
            lhs = zT_half(ct)[:].rearrange("p (j l) -> p j l", l=8)[:, :, jl]
            nc.tensor.matmul(vps[:], lhs, m_w(ct),
                             start=(ct == 0), stop=(ct == 1))
        dst = v_perm[:].rearrange("p (h j d) -> p h j d", h=H, j=NJL)[:, :, jl, 0:DH]
        nc.vector.scalar_tensor_tensor(
            dst, vps[:].rearrange("p (h d) -> p h d", h=H), 1.0,
            m_b_bc[:].rearrange("p (h d) -> p h d", h=H), OP.mult, OP.add)

    # skip_b broadcast
    skb_ps = psc.tile([128, OUT], F32)
    nc.tensor.matmul(skb_ps[:], ones_row[:], sk_b[:], start=True, stop=True)
    skb = const.tile([128, OUT], F32)
    nc.vector.tensor_copy(skb[:], skb_ps[:])

    # ---------------- main loop ----------------
    slabp = P("slab", bufs=8)
    adjp = P("adjp", bufs=2)
    tmpp = P("tmpblk", bufs=2)
    lp = lp_pool
    pblk = P("pblock", bufs=2)
    rp = P("rasm", bufs=2)
    outp = P("outs", bufs=2)

    att1v = att1bc[:].rearrange("p (i h) -> p i h", h=H)
    att2v = att2g[:].rearrange("p (h j) -> p h j", h=H)
    for it in range(n_iters):
        for ib in range(NBLK):
            # fused (att1 + att2 + cst) block tile [j_hi, (i128, j_lo, h)] bf16
            tmp_blk = tmpp.tile([128, 128 * 64], BF16)
            tmp3 = tmp_blk[:].rearrange("p (i j h) -> p i j h", i=128, j=NJL)
            a1blk = att1v[:, ib * 128:(ib + 1) * 128, :]
            for jl in range(NJL):
                a2s = att2v[:, :, jl].rearrange("p (x h) -> p x h", x=1)
                nc.vector.scalar_tensor_tensor(
                    tmp3[:, :, jl, :], a1blk, 1.0,
                    a2s.broadcast_to([128, 128, H]), OP.mult, OP.add)
            p_block = pblk.tile([128, 128 * 64], BF16)  # (i 128, j_lo 8, h 8)
            tmp2 = tmp_blk[:]
            for oct in range(NOCT):
                s8 = slabp.tile([128, 1024], BF16)
                nc.sync.dma_start(s8[:], d["e_oct"][ib * NOCT + oct])
                if oct % 4 == 0:
                    adjg = adjp.tile([128, 4, 64], BF16, name="adjg")
                    nc.gpsimd.dma_start(adjg[:], d["adj_oct"][ib * 4 + oct // 4])
                L = lp.tile([128, 512], F32, tag="L")
                # PSUM init: adj bias + (att1 + att2 + cst), via DVE
                adjv = adjg[:, oct % 4, :].rearrange("p (c x) -> p c x", x=1)
                nc.vector.scalar_tensor_tensor(
                    L[:].rearrange("p (c h) -> p c h", h=H),
                    adjv.broadcast_to([128, 64, H]), 1.0,
                    tmp2[:, oct * 512:(oct + 1) * 512]
                    .rearrange("p (c h) -> p c h", h=H),
                    OP.mult, OP.add)
                # att_e per i, accumulated on PE
                for il in range(8):
                    nc.tensor.matmul(L[:, il * 64:(il + 1) * 64],
                                     s8[:, il * 128:(il + 1) * 128], bd[:],
                                     start=False, stop=(il == 7),
                                     skip_group_check=True)
                # leaky relu in place (PSUM), then exp -> bf16 P block
                nc.scalar.activation(L[:], L[:], AF.Prelu, alpha=0.01)
                nc.scalar.activation(
                    p_block[:, oct * 512:(oct + 1) * 512], L[:], AF.Exp)

            # attention @ V for this block
            r_asm = rp.tile([128, OUT], F32)
            pb4 = p_block[:].rearrange("p (i j h) -> p i j h", i=128, j=NJL)
            vp4 = v_perm[:].rearrange("p (h j d) -> p h j d", h=H, j=NJL)
            for h in range(H):
                av = psc.tile([128, DH + 1], F32)
                for jl in range(NJL):
                    nc.tensor.matmul(av[:], pb4[:, :, jl, h], vp4[:, h, jl, :],
                                     start=(jl == 0), stop=(jl == 7))
                recip = rp.tile([128, 1], F32)
                nc.vector.reciprocal(recip[:], av[:, DH:DH + 1])
                nc.vector.tensor_scalar_mul(
                    r_asm[:, h * DH:(h + 1) * DH], av[:, 0:DH], recip[:])

            # skip connection + relu + store
            sk = psc.tile([128, OUT], F32)
            for ct in range(2):
                lhs = zT_half(ct)[:, OWN_I0 + ib * 128:OWN_I0 + (ib + 1) * 128]
                nc.tensor.matmul(sk[:], lhs, sk_w(ct),
                                 start=(ct == 0), stop=False,
                                 skip_group_check=True)
            nc.tensor.matmul(sk[:], ones_row[:], sk_b[:], start=False, stop=True,
                             skip_group_check=True)
            nc.vector.scalar_tensor_tensor(sk[:], sk[:], 1.0, r_asm[:],
                                           OP.mult, OP.add)
            ob = outp.tile([128, OUT], F32)
            nc.scalar.activation(ob[:], sk[:], AF.Relu)
            nc.gpsimd.dma_start(ret[ib * 128:(ib + 1) * 128, :], ob[:])


OWN_I0 = 0  # own rows always at z columns 0..511 (inputs pre-rotated)


def split_multi_waits(nc):
    """Walrus codegen limits sem-waits per instruction (1 on Drain, ~2 on
    others). Hoist extras onto preceding wait-only NoOps on the same engine."""
    import bass_rust
    for fn in nc.m.functions:
        for bb in fn.blocks:
            out = []
            for inst in bb.instructions:
                si = inst.sync_info
                waits = list(si.on_wait) if si is not None else []
                limit = 1
                if len(waits) > limit:
                    extra, keep = waits[:-limit], waits[-limit:]
                    for i in range(len(extra)):
                        nop = mybir.InstNoOp(
                            name=nc.get_next_instruction_name(), ins=[], outs=[])
                        nop.engine = inst.engine
                        nop.sync_info = bass_rust.SyncInfo(
                            on_wait=[extra[i]], on_update=[])
                        nc.register_instruction(nop)
                        out.append(nop)
                    inst.sync_info = bass_rust.SyncInfo(
                        on_wait=keep, on_update=list(si.on_update))
                out.append(inst)
            bb.instructions[:] = out


def shard_inputs(inputs):
    """Full inputs -> list of 8 per-core in_maps (numpy)."""
    import ml_dtypes
    BF = ml_dtypes.bfloat16
    e = np.ascontiguousarray(inputs["edge_fts"], dtype=np.float32)
    nf = np.ascontiguousarray(inputs["node_fts"], dtype=np.float32)
    hd = np.ascontiguousarray(inputs["hidden"], dtype=np.float32)
    gfa = np.ascontiguousarray(inputs["graph_fts"], dtype=np.float32)
    adj = np.asarray(inputs["adj_mat"])
    w = {k: np.ascontiguousarray(inputs[k], dtype=np.float32) for k in (
        "m_w", "m_b", "skip_w", "skip_b", "a1_w", "a1_b", "a2_w", "a2_b",
        "ae_w", "ae_b", "ag_w", "ag_b")}
    maps = []
    for c in range(NC):
        b, ih = c // 2, c % 2
        i0 = ih * NI
        # For odd cores, rotate the j axis (and z rows) by -512 so that the
        # core's own rows always sit at z columns 0..511. The attention sum
        # over j is permutation-invariant, so rolling e/adj/z consistently
        # leaves the output unchanged.
        ej = e[b, i0:i0 + NI]
        aj = adj[b, i0:i0 + NI, :]
        nfb, hdb = nf[b], hd[b]
        if ih == 1:
            ej = np.roll(ej, -NI, axis=1)
            aj = np.roll(aj, -NI, axis=1)
            nfb = np.roll(nfb, -NI, axis=0)
            hdb = np.roll(hdb, -NI, axis=0)
        # e_oct[oct][(j_lo,e)=128, (i8, j_hi128)=1024] bf16, octet-contiguous
        # ej: [512 i, 1024 j, 16 e] -> [64 oct, 8 i, 128 j_hi, 8 jl, 16 e]
        eo = ej.reshape(64, 8, 128, 8, 16).transpose(0, 3, 4, 1, 2)
        e_oct = np.ascontiguousarray(eo.reshape(64, 128, 1024), dtype=BF)
        # adj bias slabs: (adj-1)*1e9 (exactly 0 / -9.98e8 in bf16)
        # aj: [512 i, 1024 j] -> [16 grp, 128 j_hi, 4 oct, (8 i, 8 jl)]
        ab = (aj.astype(np.float32) - 1.0) * 1e9
        ag4 = ab.reshape(16, 4, 8, 128, 8).transpose(0, 3, 1, 2, 4)
        adj_oct = np.ascontiguousarray(ag4.reshape(16, 128, 4, 64), dtype=BF)
        m = {
            "e_oct": e_oct,
            "adj_oct": adj_oct,
            "nf": np.ascontiguousarray(nfb), "hd": np.ascontiguousarray(hdb),
            "gf": gfa[b].reshape(FG, 1),
            "m_w": w["m_w"], "m_b": w["m_b"].reshape(1, OUT),
            "skip_w": w["skip_w"], "skip_b": w["skip_b"].reshape(1, OUT),
            "a1_w": w["a1_w"], "a1_b": w["a1_b"].reshape(1, H),
            "a2_w": w["a2_w"], "a2_b": w["a2_b"].reshape(1, H),
            "ae_w": w["ae_w"], "ae_b": w["ae_b"].reshape(1, H),
            "ag_w": w["ag_w"], "ag_b": w["ag_b"].reshape(1, H),
        }
        maps.append(m)
    return maps


def build(n_iters=1):
    """One program shared by all 8 cores (inputs are pre-rotated so own
    rows always sit at z columns 0..511)."""
    nc = bass.Bass("TRN2", target_bir_lowering=False, debug=False,
                   num_devices=NC)
    build_core_program(nc, n_iters=n_iters)
    split_multi_waits(nc)
    return nc


def kernel(**inputs):
    from concourse.bass_utils import run_bass_kernel_spmd
    maps = shard_inputs(inputs)
    nc = build(n_iters=1)
    res = run_bass_kernel_spmd(nc, maps, list(range(NC))).results
    out = np.zeros((B, N, OUT), np.float32)
    for c in range(NC):
        b, ih = c // 2, c % 2
        out[b, ih * NI:(ih + 1) * NI] = res[c]["ret"]
    return out


# revision 50
# speedup vs baseline: 1.6062x; 1.6062x over previous
"""GAT layer Bass kernel for Trainium2, 8-core SPMD.

Sharding: core c handles batch b = c//2 and row-half ih = c%2 (512 rows of i).

Design (v4):
- e-slabs shipped as fp8e4 in group-contiguous layout (2 octets = 16 rows
  per group; 2KB descriptors; no on-chip cast). att_e is computed by PE
  matmuls against a block-diagonal ae_w (slab stationary, j on out
  partitions).
- Logit tiles L cover 2 octets: [128 j_hi, (j_lo8, i16, h8)] f32 PSUM.
- PSUM init (att1 + att2 + cst + adj bias) via DVE scalar_tensor_tensor:
  a per-block fused bf16 tile (att1+att2+cst, built one j_lo slice at a
  time, software-pipelined across the previous block) plus a per-group
  stt that adds the adjacency bias ((adj-1)*1e9 bf16 from host) and
  writes PSUM.
- All weights and zT are bf16 on-chip (bf16 transposes and 1 cyc/col
  moving operands); f32 only in PSUM accumulation.
"""
import os
import sys
sys.path.insert(0, "/opt/trn_rl_repo")
# Whole-tile dependency tracking: subtile (strided-view) intersection
# misses some DVE->PE orderings in this kernel and races on HW.
os.environ.setdefault("BY_DEFAULT_DISABLE_SUBTILE_DEPS", "1")
from contextlib import ExitStack

import numpy as np

import concourse.bass as bass
import concourse.tile as tile
from concourse import mybir
from concourse.masks import make_identity

F32 = mybir.dt.float32
BF16 = mybir.dt.bfloat16
E8 = mybir.dt.float8e4
AF = mybir.ActivationFunctionType
OP = mybir.AluOpType

B, N = 4, 1024
FN, FH, FE, FG = 128, 128, 16, 128
OUT, H = 128, 8
DH = OUT // H          # 16
ZIN = FN + FH          # 256
NC = 8                 # cores
NI = N // 2            # own rows per core = 512
NJH, NJL = N // 8, 8   # j = j_hi*8 + j_lo
NBLK = NI // 128       # i-blocks per core = 4
NOCT = 128 // 8        # octets per block = 16
NGRP = NOCT // 2       # 2-octet groups per block = 8


def build_core_program(nc, n_iters=1):
    d = {}
    def inp(name, shape, dtype=F32):
        d[name] = nc.dram_tensor(name, shape, dtype, kind="ExternalInput").ap()
    inp("e_oct", [NI // 16, 128, 2048], E8)    # [grp, (j_lo,e), (i16, j_hi)]
    inp("adj_g", [NBLK * 2, 128, 512], BF16)   # [gp4, j_hi, (4grp, i16, jl8)]
    inp("nf", [N, FN])
    inp("hd", [N, FH])
    inp("gf", [FG, 1])
    inp("m_w", [ZIN, OUT]); inp("m_b", [1, OUT])
    inp("skip_w", [ZIN, OUT]); inp("skip_b", [1, OUT])
    inp("a1_w", [ZIN, H]); inp("a1_b", [1, H])
    inp("a2_w", [ZIN, H]); inp("a2_b", [1, H])
    inp("ae_w", [FE, H]); inp("ae_b", [1, H])
    inp("ag_w", [FG, H]); inp("ag_b", [1, H])
    ret = nc.dram_tensor("ret", [NI, OUT], F32, kind="ExternalOutput").ap()

    with tile.TileContext(nc) as tc:
        with ExitStack() as ctx:
            emit(ctx, tc, d, ret, n_iters)


def emit(ctx, tc, d, ret, n_iters):
    nc = tc.nc
    P = lambda name, bufs=1: ctx.enter_context(tc.tile_pool(name=name, bufs=bufs))
    PS = lambda name, bufs=1: ctx.enter_context(
        tc.tile_pool(name=name, bufs=bufs, space="PSUM"))

    const = P("const")          # long-lived constants / staging
    psc_pool = PS("ps_small", bufs=2)   # [128, <=128] psum tiles, shared slots
    lp_pool = PS("logits", bufs=3)      # [128, 1024] f32 logit tiles (2 banks)
    def psc(shape):
        return psc_pool.tile(shape, F32, tag="s", name="pstile")

    # ---------------- prologue: constants ----------------
    ident = const.tile([128, 128], BF16)
    make_identity(nc, ident[:])
    ones_bf = const.tile([128, 128], BF16)
    nc.gpsimd.memset(ones_bf[:], 1.0)
    ones_row = const.tile([1, 128], BF16)
    nc.gpsimd.memset(ones_row[:], 1.0)

    # small weights into sbuf (bf16 via gpsimd cast-dma);
    # ZIN-row weights stored as [128, (2, n)]
    wpool = P("weights")
    def load(name, shape, dtype=BF16):
        t = wpool.tile(shape, dtype, name=name)
        nc.gpsimd.dma_start(t[:], d[name][:])
        return t
    def load2(name, ncols):
        t = wpool.tile([128, 2, ncols], BF16, name=name)
        nc.gpsimd.dma_start(t[:], d[name][:].rearrange("(c p) n -> p c n", c=2))
        return lambda ct: t[:, ct, :]
    m_w = load2("m_w", OUT);  m_b = load("m_b", [1, OUT])
    sk_w = load2("skip_w", OUT); sk_b = load("skip_b", [1, OUT])
    a1_w = load2("a1_w", H); a1_b = load("a1_b", [1, H], F32)
    a2_w = load2("a2_w", H); a2_b = load("a2_b", [1, H], F32)
    ae_w = load("ae_w", [FE, H]); ae_b = load("ae_b", [1, H], F32)
    ag_w = load("ag_w", [FG, H]); ag_b = load("ag_b", [1, H], F32)
    gf = load("gf", [FG, 1])

    # blockdiag bd[(j_lo,e), (j_lo', h)] = ae_w[e,h] * (j_lo == j_lo')
    bd = const.tile([128, 64], BF16)
    nc.gpsimd.memset(bd[:], 0.0)
    for jl in range(NJL):
        dst = bd[:].rearrange("p (j h) -> p j h", j=NJL)[jl * 16:(jl + 1) * 16, jl, :]
        nc.gpsimd.dma_start(dst, ae_w[:, :])

    # zT: [c, j] two c-tiles of [128, 1024] bf16
    zT = const.tile([128, 2 * N], BF16)  # cols [0:1024]=nf.T, [1024:2048]=hd.T
    zpool = P("zstage", bufs=3)
    for half, src in ((0, d["nf"]), (1, d["hd"])):
        for jb in range(N // 128):
            st = zpool.tile([128, 128], BF16)
            nc.gpsimd.dma_start(st[:], src[jb * 128:(jb + 1) * 128, :])
            tp = psc_pool.tile([128, 128], BF16, tag="s", name="tptile")
            nc.tensor.transpose(tp[:], st[:], ident[:])
            nc.vector.tensor_copy(
                zT[:, half * N + jb * 128: half * N + (jb + 1) * 128], tp[:])

    def zT_half(h_idx):
        return zT[:, h_idx * N:(h_idx + 1) * N]

    # cst[h] = a1_b + a2_b + ae_b + ag_b + gf @ ag_w   (shape [1, 8])
    attg_ps = psc_pool.tile([1, H], F32, tag="s", name="attg_ps")
    nc.tensor.matmul(attg_ps[:], gf[:], ag_w[:], start=True, stop=True)
    cstv = const.tile([1, H], F32)
    nc.vector.scalar_tensor_tensor(cstv[:], a1_b[:], 1.0, a2_b[:], OP.mult, OP.add)
    nc.vector.scalar_tensor_tensor(cstv[:], cstv[:], 1.0, ae_b[:], OP.mult, OP.add)
    nc.vector.scalar_tensor_tensor(cstv[:], cstv[:], 1.0, ag_b[:], OP.mult, OP.add)
    nc.vector.scalar_tensor_tensor(cstv[:], cstv[:], 1.0, attg_ps[:], OP.mult, OP.add)
    cstv_bf = const.tile([1, H], BF16)
    nc.vector.tensor_copy(cstv_bf[:], cstv[:])
    # broadcast cst to all 128 partitions: ones_row.T @ cstv
    cstb_ps = psc([128, H])
    nc.tensor.matmul(cstb_ps[:], ones_row[:], cstv_bf[:], start=True, stop=True)
    cstb = const.tile([128, H], F32)
    nc.vector.tensor_copy(cstb[:], cstb_ps[:])

    # att2g[j_hi, (j_lo, h)] = att_2[j, h] + cst[h]   (f32 [128, 64])
    att2g = const.tile([128, 64], F32)
    for jl in range(NJL):
        a2ps = psc([128, H])
        for ct in range(2):
            lhs = zT_half(ct)[:].rearrange("p (j l) -> p j l", l=8)[:, :, jl]
            nc.tensor.matmul(a2ps[:], lhs, a2_w(ct),
                             start=(ct == 0), stop=(ct == 1))
        dst = att2g[:].rearrange("p (j h) -> p j h", j=NJL)[:, jl, :]
        nc.vector.scalar_tensor_tensor(dst, a2ps[:], 1.0, cstb[:], OP.mult, OP.add)

    # q_sum[k, (i, h)] bf16: folded z (x) a1_w product so that
    # ones.T @ q_sum = att_1[i, h] broadcast over all partitions.
    q_sum = const.tile([128, NI * H], BF16)
    qtmp = const.tile([128, NI * H], F32)
    for ct in range(2):
        zslice = zT_half(ct)[:, OWN_I0:OWN_I0 + NI]
        z3 = zslice.rearrange("p (i x) -> p i x", x=1).broadcast_to([128, NI, H])
        a3 = a1_w(ct).rearrange("p (x h) -> p x h", x=1).broadcast_to([128, NI, H])
        if ct == 0:
            nc.vector.scalar_tensor_tensor(
                qtmp[:].rearrange("p (i h) -> p i h", h=H), z3, 1.0, a3,
                OP.mult, OP.mult)
        else:
            q2 = const.tile([128, NI * H], F32)
            nc.vector.scalar_tensor_tensor(
                q2[:].rearrange("p (i h) -> p i h", h=H), z3, 1.0, a3,
                OP.mult, OP.mult)
            nc.vector.scalar_tensor_tensor(
                q_sum[:].rearrange("p (i h) -> p i h", h=H),
                qtmp[:].rearrange("p (i h) -> p i h", h=H), 1.0,
                q2[:].rearrange("p (i h) -> p i h", h=H), OP.mult, OP.add)

    # att1bc[p, (i, h)] bf16 [128, 4096]: att_1 broadcast over partitions
    att1bc = const.tile([128, NI * H], BF16)
    for hb in range(NI * H // 512):
        bps = lp_pool.tile([128, 512], F32, tag="L")
        nc.tensor.matmul(bps[:], ones_bf[:],
                         q_sum[:, hb * 512:(hb + 1) * 512],
                         start=True, stop=True)
        nc.scalar.copy(att1bc[:, hb * 512:(hb + 1) * 512], bps[:])

    # V_perm[j_hi, (h, j_lo, 17)] bf16; col 16 of each (h,j_lo) group is 1.0
    v_perm = const.tile([128, H * NJL * (DH + 1)], BF16)
    nc.gpsimd.memset(v_perm[:], 1.0)
    m_b_bc_ps = psc([128, OUT])
    nc.tensor.matmul(m_b_bc_ps[:], ones_row[:], m_b[:], start=True, stop=True)
    m_b_bc = const.tile([128, OUT], F32)
    nc.vector.tensor_copy(m_b_bc[:], m_b_bc_ps[:])
    for jl in range(NJL):
        vps = psc([128, OUT])
        for ct in range(2):
            lhs = zT_half(ct)[:].rearrange("p (j l) -> p j l", l=8)[:, :, jl]
            nc.tensor.matmul(vps[:], lhs, m_w(ct),
                             start=(ct == 0), stop=(ct == 1))
        dst = v_perm[:].rearrange("p (h j d) -> p h j d", h=H, j=NJL)[:, :, jl, 0:DH]
        nc.vector.scalar_tensor_tensor(
            dst, vps[:].rearrange("p (h d) -> p h d", h=H), 1.0,
            m_b_bc[:].rearrange("p (h d) -> p h d", h=H), OP.mult, OP.add)

    # skip_b broadcast
    skb_ps = psc([128, OUT])
    nc.tensor.matmul(skb_ps[:], ones_row[:], sk_b[:], start=True, stop=True)
    skb = const.tile([128, OUT], F32)
    nc.vector.tensor_copy(skb[:], skb_ps[:])

    # ---------------- main loop ----------------
    slabp = P("slab", bufs=6)
    adjp = P("adjp", bufs=2)
    tmpp = P("tmpblk", bufs=2)
    lp = lp_pool
    pblk = P("pblock", bufs=2)
    rp = P("rasm", bufs=2)
    outp = P("outs", bufs=2)

    att1v = att1bc[:].rearrange("p (i h) -> p i h", h=H)
    att2v = att2g[:].rearrange("p (j h) -> p j h", j=NJL)
    vp4 = v_perm[:].rearrange("p (h j d) -> p h j d", h=H, j=NJL)

    # tmp_blk cols: (i128, j_lo, h) — per-block fused (att1+att2+cst) bf16
    tmp_tiles = {}
    def emit_stt1(key, ib, jl):
        if key not in tmp_tiles:
            tmp_tiles[key] = tmpp.tile([128, 128 * 64], BF16, name="tmp_blk",
                                       tag="tmp_blk")
        tmp3 = tmp_tiles[key][:].rearrange("p (i j h) -> p i j h",
                                           i=128, j=NJL)
        a1blk = att1v[:, ib * 128:(ib + 1) * 128, :]
        a2s = att2v[:, jl, :].rearrange("p (x h) -> p x h", x=1)
        nc.vector.scalar_tensor_tensor(
            tmp3[:, :, jl, :], a1blk, 1.0,
            a2s.broadcast_to([128, 128, H]), OP.mult, OP.add)

    def emit_av_h(pb4, r_asm, h):
        av = psc([128, DH + 1])
        for jl in range(NJL):
            nc.tensor.matmul(av[:], pb4[:, :, jl, h], vp4[:, h, jl, :],
                             start=(jl == 0), stop=(jl == 7))
        recip = rp.tile([128, 1], F32)
        nc.vector.reciprocal(recip[:], av[:, DH:DH + 1])
        nc.vector.tensor_scalar_mul(
            r_asm[:, h * DH:(h + 1) * DH], av[:, 0:DH], recip[:])

    def emit_block_tail(ib, r_asm):
        # skip connection + relu + store
        sk = psc([128, OUT])
        for ct in range(2):
            lhs = zT_half(ct)[:, OWN_I0 + ib * 128:OWN_I0 + (ib + 1) * 128]
            nc.tensor.matmul(sk[:], lhs, sk_w(ct),
                             start=(ct == 0), stop=False,
                             skip_group_check=True)
        nc.tensor.matmul(sk[:], ones_row[:], sk_b[:], start=False, stop=True,
                         skip_group_check=True)
        nc.vector.scalar_tensor_tensor(sk[:], sk[:], 1.0, r_asm[:],
                                       OP.mult, OP.add)
        ob = outp.tile([128, OUT], F32)
        nc.scalar.activation(ob[:], sk[:], AF.Relu)
        nc.gpsimd.dma_start(ret[ib * 128:(ib + 1) * 128, :], ob[:])

    TB = n_iters * NBLK
    prev = None   # (pb4, r_asm, ib) awaiting interleaved A@V
    for jl in range(NJL):
        emit_stt1(0, 0, jl)
    for t in range(TB):
        ib = t % NBLK
        tmp_blk = tmp_tiles.pop(t)
        p_block = pblk.tile([128, 64 * 128], BF16)  # (i128, j_lo, h)
        pb4 = p_block[:].rearrange("p (i j h) -> p i j h", i=128, j=NJL)
        for g in range(NGRP):   # group = 2 octets = 16 i; L = 2 PSUM banks
            s8 = slabp.tile([128, 2048], E8)
            nc.sync.dma_start(s8[:], d["e_oct"][ib * NGRP + g])
            if g % 4 == 0:
                adjg = adjp.tile([128, 4, 128], BF16, name="adjg")
                nc.gpsimd.dma_start(
                    adjg[:], d["adj_g"][ib * 2 + g // 4]
                    .rearrange("p (o c) -> p o c", o=4))
            L = lp.tile([128, 1024], F32, tag="L")
            # fuse adj bias + (att1 + att2 + cst) on DVE into SBUF bf16
            init_g = tmpp.tile([128, 1024], BF16, name="init_g", tag="init_g")
            adjv = adjg[:, g % 4, :].rearrange("p (c x) -> p c x", x=1)
            nc.vector.scalar_tensor_tensor(
                init_g[:].rearrange("p (c h) -> p c h", h=H),
                adjv.broadcast_to([128, 128, H]), 1.0,
                tmp_blk[:, g * 1024:(g + 1) * 1024]
                .rearrange("p (c h) -> p c h", h=H),
                OP.mult, OP.add)
            # PSUM init via PE (identity matmul, one per bank), then att_e
            # per i accumulated on PE (contiguous 64-col outs)
            for bank in range(2):
                nc.tensor.matmul(L[:, bank * 512:(bank + 1) * 512], ident[:],
                                 init_g[:, bank * 512:(bank + 1) * 512],
                                 start=True, stop=False, skip_group_check=True)
            for il in range(16):
                nc.tensor.matmul(L[:, il * 64:(il + 1) * 64],
                                 s8[:, il * 128:(il + 1) * 128], bd[:],
                                 start=False, stop=(il == 15),
                                 skip_group_check=True)
            # leaky relu in place (PSUM), then exp -> bf16 P block
            nc.scalar.activation(L[:], L[:], AF.Prelu, alpha=0.01)
            nc.scalar.activation(
                p_block[:, g * 1024:(g + 1) * 1024], L[:], AF.Exp)
            # software pipelining: next block's fused tile, one slice per
            # group, and the previous block's attention@V, one head per group
            if t + 1 < TB:
                emit_stt1(t + 1, (t + 1) % NBLK, g)
            if prev is not None:
                emit_av_h(prev[0], prev[1], g)
        if prev is not None:
            emit_block_tail(prev[2], prev[1])
        prev = (pb4, rp.tile([128, OUT], F32, name="r_asm", tag="r_asm"), ib)
    for h in range(H):
        emit_av_h(prev[0], prev[1], h)
    emit_block_tail(prev[2], prev[1])


OWN_I0 = 0  # own rows always at z columns 0..511 (inputs pre-rotated)


def split_multi_waits(nc):
    """Walrus codegen limits sem-waits per instruction (1 on Drain, ~2 on
    others). Hoist extras onto preceding wait-only NoOps on the same engine."""
    import bass_rust
    for fn in nc.m.functions:
        for bb in fn.blocks:
            out = []
            for inst in bb.instructions:
                si = inst.sync_info
                waits = list(si.on_wait) if si is not None else []
                limit = 1
                if len(waits) > limit:
                    extra, keep = waits[:-limit], waits[-limit:]
                    for i in range(len(extra)):
                        nop = mybir.InstNoOp(
                            name=nc.get_next_instruction_name(), ins=[], outs=[])
                        nop.engine = inst.engine
                        nop.sync_info = bass_rust.SyncInfo(
                            on_wait=[extra[i]], on_update=[])
                        nc.register_instruction(nop)
                        out.append(nop)
                    inst.sync_info = bass_rust.SyncInfo(
                        on_wait=keep, on_update=list(si.on_update))
                out.append(inst)
            bb.instructions[:] = out


def shard_inputs(inputs):
    """Full inputs -> list of 8 per-core in_maps (numpy)."""
    import ml_dtypes
    BF = ml_dtypes.bfloat16
    E8NP = np.dtype(mybir.dt.np(mybir.dt.float8e4))
    e = np.ascontiguousarray(inputs["edge_fts"], dtype=np.float32)
    nf = np.ascontiguousarray(inputs["node_fts"], dtype=np.float32)
    hd = np.ascontiguousarray(inputs["hidden"], dtype=np.float32)
    gfa = np.ascontiguousarray(inputs["graph_fts"], dtype=np.float32)
    adj = np.asarray(inputs["adj_mat"])
    w = {k: np.ascontiguousarray(inputs[k], dtype=np.float32) for k in (
        "m_w", "m_b", "skip_w", "skip_b", "a1_w", "a1_b", "a2_w", "a2_b",
        "ae_w", "ae_b", "ag_w", "ag_b")}
    maps = []
    for c in range(NC):
        b, ih = c // 2, c % 2
        i0 = ih * NI
        # For odd cores, rotate the j axis (and z rows) by -512 so that the
        # core's own rows always sit at z columns 0..511. The attention sum
        # over j is permutation-invariant, so rolling e/adj/z consistently
        # leaves the output unchanged.
        ej = e[b, i0:i0 + NI]
        aj = adj[b, i0:i0 + NI, :]
        nfb, hdb = nf[b], hd[b]
        if ih == 1:
            ej = np.roll(ej, -NI, axis=1)
            aj = np.roll(aj, -NI, axis=1)
            nfb = np.roll(nfb, -NI, axis=0)
            hdb = np.roll(hdb, -NI, axis=0)
        # e_oct[grp][(j_lo,e)=128, (i16, j_hi128)=2048] fp8, grp-contiguous
        # ej: [512 i, 1024 j, 16 e] -> [32 grp, 16 i, 128 j_hi, 8 jl, 16 e]
        eo = ej.reshape(32, 16, 128, 8, 16).transpose(0, 3, 4, 1, 2)
        e_oct = np.ascontiguousarray(eo.reshape(32, 128, 2048).astype(E8NP))
        # adj bias slabs: (adj-1)*1e9, cols (i16, j_lo8) per group,
        # 4 groups (=1/2 block) per row: [8, 128 j_hi, (4 grp, 16 i, 8 jl)]
        ab = (aj.astype(np.float32) - 1.0) * 1e9
        # [512 i, 1024 j] -> [8 gp4, 4 grp, 16 i, 128 j_hi, 8 jl]
        a5 = ab.reshape(8, 4, 16, 128, 8).transpose(0, 3, 1, 2, 4)
        adj_g = np.ascontiguousarray(a5.reshape(8, 128, 512), dtype=BF)
        m = {
            "e_oct": e_oct,
            "adj_g": adj_g,
            "nf": np.ascontiguousarray(nfb), "hd": np.ascontiguousarray(hdb),
            "gf": gfa[b].reshape(FG, 1),
            "m_w": w["m_w"], "m_b": w["m_b"].reshape(1, OUT),
            "skip_w": w["skip_w"], "skip_b": w["skip_b"].reshape(1, OUT),
            "a1_w": w["a1_w"], "a1_b": w["a1_b"].reshape(1, H),
            "a2_w": w["a2_w"], "a2_b": w["a2_b"].reshape(1, H),
            "ae_w": w["ae_w"], "ae_b": w["ae_b"].reshape(1, H),
            "ag_w": w["ag_w"], "ag_b": w["ag_b"].reshape(1, H),
        }
        maps.append(m)
    return maps


def build(n_iters=1):
    """One program shared by all 8 cores (inputs are pre-rotated so own
    rows always sit at z columns 0..511)."""
    nc = bass.Bass("TRN2", target_bir_lowering=False, debug=False,
                   num_devices=NC)
    build_core_program(nc, n_iters=n_iters)
    split_multi_waits(nc)
    return nc


def kernel(**inputs):
    from concourse.bass_utils import run_bass_kernel_spmd
    maps = shard_inputs(inputs)
    nc = build(n_iters=1)
    res = run_bass_kernel_spmd(nc, maps, list(range(NC))).results
    out = np.zeros((B, N, OUT), np.float32)
    for c in range(NC):
        b, ih = c // 2, c % 2
        out[b, ih * NI:(ih + 1) * NI] = res[c]["ret"]
    return out


# revision 54
# speedup vs baseline: 1.8993x; 1.1824x over previous
"""GAT layer Bass kernel for Trainium2, 8-core SPMD.

Sharding: core c handles batch b = c//2 and row-half ih = c%2 (512 rows of i).

Design (v4):
- e-slabs shipped as fp8e4 in group-contiguous layout (2 octets = 16 rows
  per group; 2KB descriptors; no on-chip cast). att_e is computed by PE
  matmuls against a block-diagonal ae_w (slab stationary, j on out
  partitions).
- Logit tiles L cover 2 octets: [128 j_hi, (j_lo8, i16, h8)] f32 PSUM.
- PSUM init (att1 + att2 + cst + adj bias) via DVE scalar_tensor_tensor:
  a per-block fused bf16 tile (att1+att2+cst, built one j_lo slice at a
  time, software-pipelined across the previous block) plus a per-group
  stt that adds the adjacency bias ((adj-1)*1e9 bf16 from host) and
  writes PSUM.
- All weights and zT are bf16 on-chip (bf16 transposes and 1 cyc/col
  moving operands); f32 only in PSUM accumulation.
"""
import os
import sys
sys.path.insert(0, "/opt/trn_rl_repo")
# Whole-tile dependency tracking: subtile (strided-view) intersection
# misses some DVE->PE orderings in this kernel and races on HW.
os.environ.setdefault("BY_DEFAULT_DISABLE_SUBTILE_DEPS", "1")
from contextlib import ExitStack

import numpy as np

import concourse.bass as bass
import concourse.tile as tile
from concourse import mybir
from concourse.masks import make_identity

F32 = mybir.dt.float32
BF16 = mybir.dt.bfloat16
E8 = mybir.dt.float8e4
AF = mybir.ActivationFunctionType
OP = mybir.AluOpType

B, N = 4, 1024
FN, FH, FE, FG = 128, 128, 16, 128
OUT, H = 128, 8
DH = OUT // H          # 16
ZIN = FN + FH          # 256
NC = 8                 # cores
NI = N // 2            # own rows per core = 512
NJH, NJL = N // 8, 8   # j = j_hi*8 + j_lo
NBLK = NI // 128       # i-blocks per core = 4
NOCT = 128 // 8        # octets per block = 16
NGRP = NOCT // 2       # 2-octet groups per block = 8


def build_core_program(nc, n_iters=1):
    d = {}
    def inp(name, shape, dtype=F32):
        d[name] = nc.dram_tensor(name, shape, dtype, kind="ExternalInput").ap()
    inp("e_oct", [NI // 16, 128, 2048], E8)    # [grp, (j_lo,e), (i16, j_hi)]
    inp("adj_g", [NBLK * 2, 128, 512], BF16)   # [gp4, j_hi, (4grp, i16, jl8)]
    inp("zT", [128, 2 * N], BF16)              # [c, (nf.T | hd.T)] bf16
    inp("bd", [128, 64], BF16)                 # blockdiag ae_w
    inp("gf", [FG, 1], BF16)
    inp("m_w", [128, 2 * OUT], BF16); inp("m_b", [1, OUT], BF16)
    inp("skip_w", [128, 2 * OUT], BF16); inp("skip_b", [1, OUT], BF16)
    inp("a1_w", [128, 2 * H], BF16); inp("a1_b", [1, H])
    inp("a2_w", [128, 2 * H], BF16); inp("a2_b", [1, H])
    inp("ae_b", [1, H])
    inp("ag_w", [FG, H], BF16); inp("ag_b", [1, H])
    ret = nc.dram_tensor("ret", [NI, OUT], F32, kind="ExternalOutput").ap()

    with tile.TileContext(nc) as tc:
        with ExitStack() as ctx:
            emit(ctx, tc, d, ret, n_iters)


def emit(ctx, tc, d, ret, n_iters):
    nc = tc.nc
    P = lambda name, bufs=1: ctx.enter_context(tc.tile_pool(name=name, bufs=bufs))
    PS = lambda name, bufs=1: ctx.enter_context(
        tc.tile_pool(name=name, bufs=bufs, space="PSUM"))

    const = P("const")          # long-lived constants / staging
    psc_pool = PS("ps_small", bufs=2)   # [128, <=128] psum tiles, shared slots
    lp_pool = PS("logits", bufs=3)      # [128, 1024] f32 logit tiles (2 banks)
    def psc(shape):
        return psc_pool.tile(shape, F32, tag="s", name="pstile")

    # ---------------- prologue: constants ----------------
    ident = const.tile([128, 128], BF16)
    make_identity(nc, ident[:])
    ones_bf = const.tile([128, 128], BF16)
    nc.gpsimd.memset(ones_bf[:], 1.0)
    ones_row = const.tile([1, 128], BF16)
    nc.gpsimd.memset(ones_row[:], 1.0)

    # small weights into sbuf: all pre-cast bf16 on host, HWDGE loads
    wpool = P("weights")
    def load(name, shape, dtype=BF16):
        t = wpool.tile(shape, dtype, name=name)
        nc.sync.dma_start(t[:], d[name][:])
        return t
    def load2(name, ncols):
        t = wpool.tile([128, 2, ncols], BF16, name=name)
        nc.sync.dma_start(t[:], d[name][:].rearrange("p (c n) -> p c n", c=2))
        return lambda ct: t[:, ct, :]
    zT = load("zT", [128, 2 * N])
    a1_w = load2("a1_w", H); a2_w = load2("a2_w", H)
    gf = load("gf", [FG, 1]); ag_w = load("ag_w", [FG, H])
    a1_b = load("a1_b", [1, H], F32); a2_b = load("a2_b", [1, H], F32)
    ae_b = load("ae_b", [1, H], F32); ag_b = load("ag_b", [1, H], F32)
    bd = load("bd", [128, 64])
    m_w = load2("m_w", OUT);  m_b = load("m_b", [1, OUT])
    sk_w = load2("skip_w", OUT); sk_b = load("skip_b", [1, OUT])

    def zT_half(h_idx):
        return zT[:, h_idx * N:(h_idx + 1) * N]

    # cst[h] = a1_b + a2_b + ae_b + ag_b + gf @ ag_w   (shape [1, 8])
    attg_ps = psc_pool.tile([1, H], F32, tag="s", name="attg_ps")
    nc.tensor.matmul(attg_ps[:], gf[:], ag_w[:], start=True, stop=True)
    cstv = const.tile([1, H], F32)
    nc.vector.scalar_tensor_tensor(cstv[:], a1_b[:], 1.0, a2_b[:], OP.mult, OP.add)
    nc.vector.scalar_tensor_tensor(cstv[:], cstv[:], 1.0, ae_b[:], OP.mult, OP.add)
    nc.vector.scalar_tensor_tensor(cstv[:], cstv[:], 1.0, ag_b[:], OP.mult, OP.add)
    nc.vector.scalar_tensor_tensor(cstv[:], cstv[:], 1.0, attg_ps[:], OP.mult, OP.add)
    cstv_bf = const.tile([1, H], BF16)
    nc.vector.tensor_copy(cstv_bf[:], cstv[:])
    # broadcast cst to all 128 partitions: ones_row.T @ cstv
    cstb_ps = psc([128, H])
    nc.tensor.matmul(cstb_ps[:], ones_row[:], cstv_bf[:], start=True, stop=True)
    cstb = const.tile([128, H], F32)
    nc.vector.tensor_copy(cstb[:], cstb_ps[:])

    # att2g[j_hi, (j_lo, h)] = att_2[j, h] + cst[h]   (f32 [128, 64])
    att2g = const.tile([128, 64], F32)
    for jl in range(NJL):
        a2ps = psc([128, H])
        for ct in range(2):
            lhs = zT_half(ct)[:].rearrange("p (j l) -> p j l", l=8)[:, :, jl]
            nc.tensor.matmul(a2ps[:], lhs, a2_w(ct),
                             start=(ct == 0), stop=(ct == 1))
        dst = att2g[:].rearrange("p (j h) -> p j h", j=NJL)[:, jl, :]
        nc.vector.scalar_tensor_tensor(dst, a2ps[:], 1.0, cstb[:], OP.mult, OP.add)

    # q_sum[k, (i, h)] bf16: folded z (x) a1_w product so that
    # ones.T @ q_sum = att_1[i, h] broadcast over all partitions.
    q_sum = const.tile([128, NI * H], BF16)
    qtmp = const.tile([128, NI * H], F32)
    for ct in range(2):
        zslice = zT_half(ct)[:, OWN_I0:OWN_I0 + NI]
        z3 = zslice.rearrange("p (i x) -> p i x", x=1).broadcast_to([128, NI, H])
        a3 = a1_w(ct).rearrange("p (x h) -> p x h", x=1).broadcast_to([128, NI, H])
        if ct == 0:
            nc.vector.scalar_tensor_tensor(
                qtmp[:].rearrange("p (i h) -> p i h", h=H), z3, 1.0, a3,
                OP.mult, OP.mult)
        else:
            q2 = const.tile([128, NI * H], F32)
            nc.vector.scalar_tensor_tensor(
                q2[:].rearrange("p (i h) -> p i h", h=H), z3, 1.0, a3,
                OP.mult, OP.mult)
            nc.vector.scalar_tensor_tensor(
                q_sum[:].rearrange("p (i h) -> p i h", h=H),
                qtmp[:].rearrange("p (i h) -> p i h", h=H), 1.0,
                q2[:].rearrange("p (i h) -> p i h", h=H), OP.mult, OP.add)

    # att1bc[p, (i, h)] bf16 [128, 4096]: att_1 broadcast over partitions
    att1bc = const.tile([128, NI * H], BF16)
    for hb in range(NI * H // 512):
        bps = lp_pool.tile([128, 512], F32, tag="L")
        nc.tensor.matmul(bps[:], ones_bf[:],
                         q_sum[:, hb * 512:(hb + 1) * 512],
                         start=True, stop=True)
        nc.scalar.copy(att1bc[:, hb * 512:(hb + 1) * 512], bps[:])

    # V_perm[j_hi, (h, j_lo, 17)] bf16; col 16 of each (h,j_lo) group is 1.0
    v_perm = const.tile([128, H * NJL * (DH + 1)], BF16)
    nc.gpsimd.memset(v_perm[:], 1.0)
    m_b_bc_ps = psc([128, OUT])
    nc.tensor.matmul(m_b_bc_ps[:], ones_row[:], m_b[:], start=True, stop=True)
    m_b_bc = const.tile([128, OUT], F32)
    nc.vector.tensor_copy(m_b_bc[:], m_b_bc_ps[:])
    for jl in range(NJL):
        vps = psc([128, OUT])
        for ct in range(2):
            lhs = zT_half(ct)[:].rearrange("p (j l) -> p j l", l=8)[:, :, jl]
            nc.tensor.matmul(vps[:], lhs, m_w(ct),
                             start=(ct == 0), stop=(ct == 1))
        dst = v_perm[:].rearrange("p (h j d) -> p h j d", h=H, j=NJL)[:, :, jl, 0:DH]
        nc.vector.scalar_tensor_tensor(
            dst, vps[:].rearrange("p (h d) -> p h d", h=H), 1.0,
            m_b_bc[:].rearrange("p (h d) -> p h d", h=H), OP.mult, OP.add)

    # skip_b broadcast
    skb_ps = psc([128, OUT])
    nc.tensor.matmul(skb_ps[:], ones_row[:], sk_b[:], start=True, stop=True)
    skb = const.tile([128, OUT], F32)
    nc.vector.tensor_copy(skb[:], skb_ps[:])

    # ---------------- main loop ----------------
    slabp = P("slab", bufs=6)
    adjp = P("adjp", bufs=2)
    tmpp = P("tmpblk", bufs=2)
    lp = lp_pool
    pblk = P("pblock", bufs=2)
    rp = P("rasm", bufs=2)
    outp = P("outs", bufs=2)

    att1v = att1bc[:].rearrange("p (i h) -> p i h", h=H)
    att2v = att2g[:].rearrange("p (j h) -> p j h", j=NJL)
    vp4 = v_perm[:].rearrange("p (h j d) -> p h j d", h=H, j=NJL)

    # tmp_blk cols: (i128, j_lo, h) — per-block fused (att1+att2+cst) bf16
    tmp_tiles = {}
    def emit_stt1(key, ib, jl):
        if key not in tmp_tiles:
            tmp_tiles[key] = tmpp.tile([128, 128 * 64], BF16, name="tmp_blk",
                                       tag="tmp_blk")
        tmp3 = tmp_tiles[key][:].rearrange("p (i j h) -> p i j h",
                                           i=128, j=NJL)
        a1blk = att1v[:, ib * 128:(ib + 1) * 128, :]
        a2s = att2v[:, jl, :].rearrange("p (x h) -> p x h", x=1)
        nc.vector.scalar_tensor_tensor(
            tmp3[:, :, jl, :], a1blk, 1.0,
            a2s.broadcast_to([128, 128, H]), OP.mult, OP.add)

    def emit_av_h(pb4, r_asm, h):
        av = psc([128, DH + 1])
        for jl in range(NJL):
            nc.tensor.matmul(av[:], pb4[:, :, jl, h], vp4[:, h, jl, :],
                             start=(jl == 0), stop=(jl == 7))
        recip = rp.tile([128, 1], F32)
        nc.vector.reciprocal(recip[:], av[:, DH:DH + 1])
        nc.vector.tensor_scalar_mul(
            r_asm[:, h * DH:(h + 1) * DH], av[:, 0:DH], recip[:])

    def emit_block_tail(ib, r_asm):
        # skip connection + relu + store
        sk = psc([128, OUT])
        for ct in range(2):
            lhs = zT_half(ct)[:, OWN_I0 + ib * 128:OWN_I0 + (ib + 1) * 128]
            nc.tensor.matmul(sk[:], lhs, sk_w(ct),
                             start=(ct == 0), stop=False,
                             skip_group_check=True)
        nc.tensor.matmul(sk[:], ones_row[:], sk_b[:], start=False, stop=True,
                         skip_group_check=True)
        nc.vector.scalar_tensor_tensor(sk[:], sk[:], 1.0, r_asm[:],
                                       OP.mult, OP.add)
        ob = outp.tile([128, OUT], F32)
        nc.scalar.activation(ob[:], sk[:], AF.Relu)
        nc.gpsimd.dma_start(ret[ib * 128:(ib + 1) * 128, :], ob[:])

    TB = n_iters * NBLK
    prev = None   # (pb4, r_asm, ib) awaiting interleaved A@V
    for jl in range(NJL):
        emit_stt1(0, 0, jl)
    for t in range(TB):
        ib = t % NBLK
        tmp_blk = tmp_tiles.pop(t)
        p_block = pblk.tile([128, 64 * 128], BF16)  # (i128, j_lo, h)
        pb4 = p_block[:].rearrange("p (i j h) -> p i j h", i=128, j=NJL)
        for g in range(NGRP):   # group = 2 octets = 16 i; L = 2 PSUM banks
            s8 = slabp.tile([128, 2048], E8)
            nc.sync.dma_start(s8[:], d["e_oct"][ib * NGRP + g])
            if g % 4 == 0:
                adjg = adjp.tile([128, 4, 128], BF16, name="adjg")
                nc.gpsimd.dma_start(
                    adjg[:], d["adj_g"][ib * 2 + g // 4]
                    .rearrange("p (o c) -> p o c", o=4))
            L = lp.tile([128, 1024], F32, tag="L")
            # fuse adj bias + (att1 + att2 + cst) on DVE into SBUF bf16
            init_g = tmpp.tile([128, 1024], BF16, name="init_g", tag="init_g")
            adjv = adjg[:, g % 4, :].rearrange("p (c x) -> p c x", x=1)
            nc.vector.scalar_tensor_tensor(
                init_g[:].rearrange("p (c h) -> p c h", h=H),
                adjv.broadcast_to([128, 128, H]), 1.0,
                tmp_blk[:, g * 1024:(g + 1) * 1024]
                .rearrange("p (c h) -> p c h", h=H),
                OP.mult, OP.add)
            # PSUM init via PE (identity matmul, one per bank), then att_e
            # per i accumulated on PE (contiguous 64-col outs)
            for bank in range(2):
                nc.tensor.matmul(L[:, bank * 512:(bank + 1) * 512], ident[:],
                                 init_g[:, bank * 512:(bank + 1) * 512],
                                 start=True, stop=False, skip_group_check=True)
            for il in range(16):
                nc.tensor.matmul(L[:, il * 64:(il + 1) * 64],
                                 s8[:, il * 128:(il + 1) * 128], bd[:],
                                 start=False, stop=(il == 15),
                                 skip_group_check=True)
            # leaky relu in place (PSUM), then exp -> bf16 P block
            nc.scalar.activation(L[:], L[:], AF.Prelu, alpha=0.01)
            nc.scalar.activation(
                p_block[:, g * 1024:(g + 1) * 1024], L[:], AF.Exp)
            # software pipelining: next block's fused tile, one slice per
            # group, and the previous block's attention@V, one head per group
            if t + 1 < TB:
                emit_stt1(t + 1, (t + 1) % NBLK, g)
            if prev is not None:
                emit_av_h(prev[0], prev[1], g)
        if prev is not None:
            emit_block_tail(prev[2], prev[1])
        prev = (pb4, rp.tile([128, OUT], F32, name="r_asm", tag="r_asm"), ib)
    for h in range(H):
        emit_av_h(prev[0], prev[1], h)
    emit_block_tail(prev[2], prev[1])


OWN_I0 = 0  # own rows always at z columns 0..511 (inputs pre-rotated)


def split_multi_waits(nc):
    """Walrus codegen limits sem-waits per instruction (1 on Drain, ~2 on
    others). Hoist extras onto preceding wait-only NoOps on the same engine."""
    import bass_rust
    for fn in nc.m.functions:
        for bb in fn.blocks:
            out = []
            for inst in bb.instructions:
                si = inst.sync_info
                waits = list(si.on_wait) if si is not None else []
                limit = 1
                if len(waits) > limit:
                    extra, keep = waits[:-limit], waits[-limit:]
                    for i in range(len(extra)):
                        nop = mybir.InstNoOp(
                            name=nc.get_next_instruction_name(), ins=[], outs=[])
                        nop.engine = inst.engine
                        nop.sync_info = bass_rust.SyncInfo(
                            on_wait=[extra[i]], on_update=[])
                        nc.register_instruction(nop)
                        out.append(nop)
                    inst.sync_info = bass_rust.SyncInfo(
                        on_wait=keep, on_update=list(si.on_update))
                out.append(inst)
            bb.instructions[:] = out


def shard_inputs(inputs):
    """Full inputs -> list of 8 per-core in_maps (numpy)."""
    import ml_dtypes
    BF = ml_dtypes.bfloat16
    E8NP = np.dtype(mybir.dt.np(mybir.dt.float8e4))
    e = np.ascontiguousarray(inputs["edge_fts"], dtype=np.float32)
    nf = np.ascontiguousarray(inputs["node_fts"], dtype=np.float32)
    hd = np.ascontiguousarray(inputs["hidden"], dtype=np.float32)
    gfa = np.ascontiguousarray(inputs["graph_fts"], dtype=np.float32)
    adj = np.asarray(inputs["adj_mat"])
    w = {k: np.ascontiguousarray(inputs[k], dtype=np.float32) for k in (
        "m_w", "m_b", "skip_w", "skip_b", "a1_w", "a1_b", "a2_w", "a2_b",
        "ae_w", "ae_b", "ag_w", "ag_b")}
    # ZIN-row weights as [128, (2, n)] bf16 (c-halves side by side)
    def w2(name, ncols):
        return np.ascontiguousarray(
            w[name].reshape(2, 128, ncols).transpose(1, 0, 2)
            .reshape(128, 2 * ncols), dtype=BF)
    # blockdiag bd[(j_lo,e), (j_lo', h)] = ae_w[e, h] * (j_lo == j_lo')
    bdm = np.zeros((8, 16, 8, H), np.float32)
    for jl in range(8):
        bdm[jl, :, jl, :] = w["ae_w"]
    bdm = np.ascontiguousarray(bdm.reshape(128, 64).astype(BF))
    maps = []
    for c in range(NC):
        b, ih = c // 2, c % 2
        i0 = ih * NI
        # For odd cores, rotate the j axis (and z rows) by -512 so that the
        # core's own rows always sit at z columns 0..511. The attention sum
        # over j is permutation-invariant, so rolling e/adj/z consistently
        # leaves the output unchanged.
        ej = e[b, i0:i0 + NI]
        aj = adj[b, i0:i0 + NI, :]
        nfb, hdb = nf[b], hd[b]
        if ih == 1:
            ej = np.roll(ej, -NI, axis=1)
            aj = np.roll(aj, -NI, axis=1)
            nfb = np.roll(nfb, -NI, axis=0)
            hdb = np.roll(hdb, -NI, axis=0)
        # e_oct[grp][(j_lo,e)=128, (i16, j_hi128)=2048] fp8, grp-contiguous
        # ej: [512 i, 1024 j, 16 e] -> [32 grp, 16 i, 128 j_hi, 8 jl, 16 e]
        eo = ej.reshape(32, 16, 128, 8, 16).transpose(0, 3, 4, 1, 2)
        e_oct = np.ascontiguousarray(eo.reshape(32, 128, 2048).astype(E8NP))
        # adj bias slabs: (adj-1)*1e9, cols (i16, j_lo8) per group,
        # 4 groups (=1/2 block) per row: [8, 128 j_hi, (4 grp, 16 i, 8 jl)]
        ab = (aj.astype(np.float32) - 1.0) * 1e9
        # [512 i, 1024 j] -> [8 gp4, 4 grp, 16 i, 128 j_hi, 8 jl]
        a5 = ab.reshape(8, 4, 16, 128, 8).transpose(0, 3, 1, 2, 4)
        adj_g = np.ascontiguousarray(a5.reshape(8, 128, 512), dtype=BF)
        m = {
            "e_oct": e_oct,
            "adj_g": adj_g,
            "zT": np.ascontiguousarray(
                np.concatenate([nfb.T, hdb.T], axis=1).astype(BF)),
            "bd": bdm,
            "gf": gfa[b].reshape(FG, 1).astype(BF),
            "m_w": w2("m_w", OUT), "m_b": w["m_b"].reshape(1, OUT).astype(BF),
            "skip_w": w2("skip_w", OUT),
            "skip_b": w["skip_b"].reshape(1, OUT).astype(BF),
            "a1_w": w2("a1_w", H), "a1_b": w["a1_b"].reshape(1, H),
            "a2_w": w2("a2_w", H), "a2_b": w["a2_b"].reshape(1, H),
            "ae_b": w["ae_b"].reshape(1, H),
            "ag_w": w["ag_w"].astype(BF),
            "ag_b": w["ag_b"].reshape(1, H),
        }
        maps.append(m)
    return maps


def build(n_iters=1):
    """One program shared by all 8 cores (inputs are pre-rotated so own
    rows always sit at z columns 0..511)."""
    nc = bass.Bass("TRN2", target_bir_lowering=False, debug=False,
                   num_devices=NC)
    build_core_program(nc, n_iters=n_iters)
    split_multi_waits(nc)
    return nc


def kernel(**inputs):
    from concourse.bass_utils import run_bass_kernel_spmd
    maps = shard_inputs(inputs)
    nc = build(n_iters=1)
    res = run_bass_kernel_spmd(nc, maps, list(range(NC))).results
    out = np.zeros((B, N, OUT), np.float32)
    for c in range(NC):
        b, ih = c // 2, c % 2
        out[b, ih * NI:(ih + 1) * NI] = res[c]["ret"]
    return out


# revision 56
# speedup vs baseline: 2.0448x; 1.0766x over previous
"""GAT layer Bass kernel for Trainium2, 8-core SPMD.

Sharding: core c handles batch b = c//2 and row-half ih = c%2 (512 rows of i).

Design (v4):
- e-slabs shipped as fp8e4 in group-contiguous layout (2 octets = 16 rows
  per group; 2KB descriptors; no on-chip cast). att_e is computed by PE
  matmuls against a block-diagonal ae_w (slab stationary, j on out
  partitions).
- Logit tiles L cover 2 octets: [128 j_hi, (j_lo8, i16, h8)] f32 PSUM.
- PSUM init (att1 + att2 + cst + adj bias) via DVE scalar_tensor_tensor:
  a per-block fused bf16 tile (att1+att2+cst, built one j_lo slice at a
  time, software-pipelined across the previous block) plus a per-group
  stt that adds the adjacency bias ((adj-1)*1e9 bf16 from host) and
  writes PSUM.
- All weights and zT are bf16 on-chip (bf16 transposes and 1 cyc/col
  moving operands); f32 only in PSUM accumulation.
"""
import os
import sys
sys.path.insert(0, "/opt/trn_rl_repo")
# Whole-tile dependency tracking: subtile (strided-view) intersection
# misses some DVE->PE orderings in this kernel and races on HW.
os.environ.setdefault("BY_DEFAULT_DISABLE_SUBTILE_DEPS", "1")
from contextlib import ExitStack

import numpy as np

import concourse.bass as bass
import concourse.tile as tile
from concourse import mybir
from concourse.masks import make_identity

F32 = mybir.dt.float32
BF16 = mybir.dt.bfloat16
E8 = mybir.dt.float8e4
AF = mybir.ActivationFunctionType
OP = mybir.AluOpType

B, N = 4, 1024
FN, FH, FE, FG = 128, 128, 16, 128
OUT, H = 128, 8
DH = OUT // H          # 16
ZIN = FN + FH          # 256
NC = 8                 # cores
NI = N // 2            # own rows per core = 512
NJH, NJL = N // 8, 8   # j = j_hi*8 + j_lo
NBLK = NI // 128       # i-blocks per core = 4
NOCT = 128 // 8        # octets per block = 16
NGRP = NOCT // 2       # 2-octet groups per block = 8


def build_core_program(nc, n_iters=1):
    d = {}
    def inp(name, shape, dtype=F32):
        d[name] = nc.dram_tensor(name, shape, dtype, kind="ExternalInput").ap()
    inp("e_oct", [NI // 16, 128, 2048], E8)    # [grp, (j_lo,e), (i16, j_hi)]
    inp("adj_g", [NBLK * 2, 128, 512], BF16)   # [gp4, j_hi, (4grp, i16, jl8)]
    inp("zT", [128, 2 * N], BF16)              # [c, (nf.T | hd.T)] bf16
    inp("bd", [128, 64], BF16)                 # blockdiag ae_w
    inp("gf", [FG, 1], BF16)
    inp("m_w", [128, 2 * OUT], BF16); inp("m_b", [1, OUT], BF16)
    inp("skip_w", [128, 2 * OUT], BF16); inp("skip_b", [1, OUT], BF16)
    inp("a1_w", [128, 2 * H], BF16); inp("a1_b", [1, H])
    inp("a2_w", [128, 2 * H], BF16); inp("a2_b", [1, H])
    inp("ae_b", [1, H])
    inp("ag_w", [FG, H], BF16); inp("ag_b", [1, H])
    ret = nc.dram_tensor("ret", [NI, OUT], F32, kind="ExternalOutput").ap()

    with tile.TileContext(nc) as tc:
        with ExitStack() as ctx:
            emit(ctx, tc, d, ret, n_iters)


def emit(ctx, tc, d, ret, n_iters):
    nc = tc.nc
    P = lambda name, bufs=1: ctx.enter_context(tc.tile_pool(name=name, bufs=bufs))
    PS = lambda name, bufs=1: ctx.enter_context(
        tc.tile_pool(name=name, bufs=bufs, space="PSUM"))

    const = P("const")          # long-lived constants / staging
    psc_pool = PS("ps_small", bufs=2)   # [128, <=128] psum tiles, shared slots
    lp_pool = PS("logits", bufs=3)      # [128, 1024] f32 logit tiles (2 banks)
    def psc(shape):
        return psc_pool.tile(shape, F32, tag="s", name="pstile")

    # ---------------- prologue: constants ----------------
    ident = const.tile([128, 128], BF16)
    make_identity(nc, ident[:])
    ones_bf = const.tile([128, 128], BF16)
    nc.gpsimd.memset(ones_bf[:], 1.0)
    ones_row = const.tile([1, 128], BF16)
    nc.gpsimd.memset(ones_row[:], 1.0)

    # small weights into sbuf: all pre-cast bf16 on host, HWDGE loads
    wpool = P("weights")
    def load(name, shape, dtype=BF16):
        t = wpool.tile(shape, dtype, name=name)
        nc.sync.dma_start(t[:], d[name][:])
        return t
    def load2(name, ncols):
        t = wpool.tile([128, 2, ncols], BF16, name=name)
        nc.sync.dma_start(t[:], d[name][:].rearrange("p (c n) -> p c n", c=2))
        return lambda ct: t[:, ct, :]
    zT = load("zT", [128, 2 * N])
    a1_w = load2("a1_w", H); a2_w = load2("a2_w", H)
    gf = load("gf", [FG, 1]); ag_w = load("ag_w", [FG, H])
    a1_b = load("a1_b", [1, H], F32); a2_b = load("a2_b", [1, H], F32)
    ae_b = load("ae_b", [1, H], F32); ag_b = load("ag_b", [1, H], F32)
    bd = load("bd", [128, 64])
    m_w = load2("m_w", OUT);  m_b = load("m_b", [1, OUT])
    sk_w = load2("skip_w", OUT); sk_b = load("skip_b", [1, OUT])

    def zT_half(h_idx):
        return zT[:, h_idx * N:(h_idx + 1) * N]

    # cst[h] = a1_b + a2_b + ae_b + ag_b + gf @ ag_w   (shape [1, 8])
    attg_ps = psc_pool.tile([1, H], F32, tag="s", name="attg_ps")
    nc.tensor.matmul(attg_ps[:], gf[:], ag_w[:], start=True, stop=True)
    cstv = const.tile([1, H], F32)
    nc.vector.scalar_tensor_tensor(cstv[:], a1_b[:], 1.0, a2_b[:], OP.mult, OP.add)
    nc.vector.scalar_tensor_tensor(cstv[:], cstv[:], 1.0, ae_b[:], OP.mult, OP.add)
    nc.vector.scalar_tensor_tensor(cstv[:], cstv[:], 1.0, ag_b[:], OP.mult, OP.add)
    nc.vector.scalar_tensor_tensor(cstv[:], cstv[:], 1.0, attg_ps[:], OP.mult, OP.add)
    cstv_bf = const.tile([1, H], BF16)
    nc.vector.tensor_copy(cstv_bf[:], cstv[:])
    # broadcast cst to all 128 partitions: ones_row.T @ cstv
    cstb_ps = psc([128, H])
    nc.tensor.matmul(cstb_ps[:], ones_row[:], cstv_bf[:], start=True, stop=True)
    cstb = const.tile([128, H], F32)
    nc.vector.tensor_copy(cstb[:], cstb_ps[:])

    # att2g[j_hi, (j_lo, h)] = att_2[j, h] + cst[h]   (f32 [128, 64])
    att2g = const.tile([128, 64], F32)
    for jl in range(NJL):
        a2ps = psc([128, H])
        for ct in range(2):
            lhs = zT_half(ct)[:].rearrange("p (j l) -> p j l", l=8)[:, :, jl]
            nc.tensor.matmul(a2ps[:], lhs, a2_w(ct),
                             start=(ct == 0), stop=(ct == 1))
        dst = att2g[:].rearrange("p (j h) -> p j h", j=NJL)[:, jl, :]
        nc.vector.scalar_tensor_tensor(dst, a2ps[:], 1.0, cstb[:], OP.mult, OP.add)

    # q_sum[k, (i, h)] bf16, per-block chunks: folded z (x) a1_w product so
    # that ones.T @ q_sum = att_1[i, h] broadcast over all partitions.
    # Chunk 0 is emitted up front so block 0 can start ~10us earlier; chunk
    # t+1 is emitted at the top of block t (before block t+1's stt1 slices).
    q_sum = const.tile([128, NI * H], BF16)
    att1bc = const.tile([128, NI * H], BF16)
    qstage = P("qstage", bufs=2)
    def emit_qchunk(blk):
        i0 = blk * 128
        qtmp = qstage.tile([128, 128 * H], F32, name="qtmp", tag="qt")
        q2 = qstage.tile([128, 128 * H], F32, name="q2", tag="q2")
        for ct in range(2):
            zslice = zT_half(ct)[:, i0:i0 + 128]
            z3 = zslice.rearrange("p (i x) -> p i x", x=1) \
                .broadcast_to([128, 128, H])
            a3 = a1_w(ct).rearrange("p (x h) -> p x h", x=1) \
                .broadcast_to([128, 128, H])
            dst = (qtmp if ct == 0 else q2)[:] \
                .rearrange("p (i h) -> p i h", h=H)
            nc.vector.scalar_tensor_tensor(dst, z3, 1.0, a3, OP.mult, OP.mult)
        nc.vector.scalar_tensor_tensor(
            q_sum[:, i0 * H:(i0 + 128) * H].rearrange("p (i h) -> p i h", h=H),
            qtmp[:].rearrange("p (i h) -> p i h", h=H), 1.0,
            q2[:].rearrange("p (i h) -> p i h", h=H), OP.mult, OP.add)
        for half in range(2):
            c0 = i0 * H + half * 512
            bps = lp_pool.tile([128, 512], F32, tag="L")
            nc.tensor.matmul(bps[:], ones_bf[:], q_sum[:, c0:c0 + 512],
                             start=True, stop=True)
            nc.scalar.copy(att1bc[:, c0:c0 + 512], bps[:])
    emit_qchunk(0)

    # V_perm[j_hi, (h, j_lo, 17)] bf16; col 16 of each (h,j_lo) group is 1.0
    v_perm = const.tile([128, H * NJL * (DH + 1)], BF16)
    nc.gpsimd.memset(v_perm[:], 1.0)
    m_b_bc_ps = psc([128, OUT])
    nc.tensor.matmul(m_b_bc_ps[:], ones_row[:], m_b[:], start=True, stop=True)
    m_b_bc = const.tile([128, OUT], F32)
    nc.vector.tensor_copy(m_b_bc[:], m_b_bc_ps[:])
    for jl in range(NJL):
        vps = psc([128, OUT])
        for ct in range(2):
            lhs = zT_half(ct)[:].rearrange("p (j l) -> p j l", l=8)[:, :, jl]
            nc.tensor.matmul(vps[:], lhs, m_w(ct),
                             start=(ct == 0), stop=(ct == 1))
        dst = v_perm[:].rearrange("p (h j d) -> p h j d", h=H, j=NJL)[:, :, jl, 0:DH]
        nc.vector.scalar_tensor_tensor(
            dst, vps[:].rearrange("p (h d) -> p h d", h=H), 1.0,
            m_b_bc[:].rearrange("p (h d) -> p h d", h=H), OP.mult, OP.add)

    # skip_b broadcast
    skb_ps = psc([128, OUT])
    nc.tensor.matmul(skb_ps[:], ones_row[:], sk_b[:], start=True, stop=True)
    skb = const.tile([128, OUT], F32)
    nc.vector.tensor_copy(skb[:], skb_ps[:])

    # ---------------- main loop ----------------
    slabp = P("slab", bufs=6)
    adjp = P("adjp", bufs=2)
    tmpp = P("tmpblk", bufs=2)
    lp = lp_pool
    pblk = P("pblock", bufs=2)
    rp = P("rasm", bufs=2)
    outp = P("outs", bufs=2)

    att1v = att1bc[:].rearrange("p (i h) -> p i h", h=H)
    att2v = att2g[:].rearrange("p (j h) -> p j h", j=NJL)
    vp4 = v_perm[:].rearrange("p (h j d) -> p h j d", h=H, j=NJL)

    # tmp_blk cols: (i128, j_lo, h) — per-block fused (att1+att2+cst) bf16
    tmp_tiles = {}
    def emit_stt1(key, ib, jl):
        if key not in tmp_tiles:
            tmp_tiles[key] = tmpp.tile([128, 128 * 64], BF16, name="tmp_blk",
                                       tag="tmp_blk")
        tmp3 = tmp_tiles[key][:].rearrange("p (i j h) -> p i j h",
                                           i=128, j=NJL)
        a1blk = att1v[:, ib * 128:(ib + 1) * 128, :]
        a2s = att2v[:, jl, :].rearrange("p (x h) -> p x h", x=1)
        nc.vector.scalar_tensor_tensor(
            tmp3[:, :, jl, :], a1blk, 1.0,
            a2s.broadcast_to([128, 128, H]), OP.mult, OP.add)

    def emit_av_h(pb4, r_asm, h):
        av = psc([128, DH + 1])
        for jl in range(NJL):
            nc.tensor.matmul(av[:], pb4[:, :, jl, h], vp4[:, h, jl, :],
                             start=(jl == 0), stop=(jl == 7))
        recip = rp.tile([128, 1], F32)
        nc.vector.reciprocal(recip[:], av[:, DH:DH + 1])
        nc.vector.tensor_scalar_mul(
            r_asm[:, h * DH:(h + 1) * DH], av[:, 0:DH], recip[:])

    def emit_block_tail(ib, r_asm):
        # skip connection + relu + store
        sk = psc([128, OUT])
        for ct in range(2):
            lhs = zT_half(ct)[:, OWN_I0 + ib * 128:OWN_I0 + (ib + 1) * 128]
            nc.tensor.matmul(sk[:], lhs, sk_w(ct),
                             start=(ct == 0), stop=False,
                             skip_group_check=True)
        nc.tensor.matmul(sk[:], ones_row[:], sk_b[:], start=False, stop=True,
                         skip_group_check=True)
        nc.vector.scalar_tensor_tensor(sk[:], sk[:], 1.0, r_asm[:],
                                       OP.mult, OP.add)
        ob = outp.tile([128, OUT], F32)
        nc.scalar.activation(ob[:], sk[:], AF.Relu)
        nc.gpsimd.dma_start(ret[ib * 128:(ib + 1) * 128, :], ob[:])

    TB = n_iters * NBLK
    prev = None   # (pb4, r_asm, ib) awaiting interleaved A@V
    for jl in range(NJL):
        emit_stt1(0, 0, jl)
    for t in range(TB):
        ib = t % NBLK
        if t + 1 < NBLK:
            emit_qchunk(t + 1)  # att1 for block t+1, before its stt1 slices
        tmp_blk = tmp_tiles.pop(t)
        p_block = pblk.tile([128, 64 * 128], BF16)  # (i128, j_lo, h)
        pb4 = p_block[:].rearrange("p (i j h) -> p i j h", i=128, j=NJL)
        for g in range(NGRP):   # group = 2 octets = 16 i; L = 2 PSUM banks
            s8 = slabp.tile([128, 2048], E8)
            nc.sync.dma_start(s8[:], d["e_oct"][ib * NGRP + g])
            if g % 4 == 0:
                adjg = adjp.tile([128, 4, 128], BF16, name="adjg")
                nc.gpsimd.dma_start(
                    adjg[:], d["adj_g"][ib * 2 + g // 4]
                    .rearrange("p (o c) -> p o c", o=4))
            L = lp.tile([128, 1024], F32, tag="L")
            # fuse adj bias + (att1 + att2 + cst) on DVE into SBUF bf16
            init_g = tmpp.tile([128, 1024], BF16, name="init_g", tag="init_g")
            adjv = adjg[:, g % 4, :].rearrange("p (c x) -> p c x", x=1)
            nc.vector.scalar_tensor_tensor(
                init_g[:].rearrange("p (c h) -> p c h", h=H),
                adjv.broadcast_to([128, 128, H]), 1.0,
                tmp_blk[:, g * 1024:(g + 1) * 1024]
                .rearrange("p (c h) -> p c h", h=H),
                OP.mult, OP.add)
            # PSUM init via PE (identity matmul, one per bank), then att_e
            # per i accumulated on PE (contiguous 64-col outs)
            for bank in range(2):
                nc.tensor.matmul(L[:, bank * 512:(bank + 1) * 512], ident[:],
                                 init_g[:, bank * 512:(bank + 1) * 512],
                                 start=True, stop=False, skip_group_check=True)
            for il in range(16):
                nc.tensor.matmul(L[:, il * 64:(il + 1) * 64],
                                 s8[:, il * 128:(il + 1) * 128], bd[:],
                                 start=False, stop=(il == 15),
                                 skip_group_check=True)
            # leaky relu in place (PSUM), then exp -> bf16 P block
            nc.scalar.activation(L[:], L[:], AF.Prelu, alpha=0.01)
            nc.scalar.activation(
                p_block[:, g * 1024:(g + 1) * 1024], L[:], AF.Exp)
            # software pipelining: next block's fused tile, one slice per
            # group, and the previous block's attention@V, one head per group
            if t + 1 < TB:
                emit_stt1(t + 1, (t + 1) % NBLK, g)
            if prev is not None:
                emit_av_h(prev[0], prev[1], g)
        if prev is not None:
            emit_block_tail(prev[2], prev[1])
        prev = (pb4, rp.tile([128, OUT], F32, name="r_asm", tag="r_asm"), ib)
    for h in range(H):
        emit_av_h(prev[0], prev[1], h)
    emit_block_tail(prev[2], prev[1])


OWN_I0 = 0  # own rows always at z columns 0..511 (inputs pre-rotated)


def split_multi_waits(nc):
    """Walrus codegen limits sem-waits per instruction (1 on Drain, ~2 on
    others). Hoist extras onto preceding wait-only NoOps on the same engine."""
    import bass_rust
    for fn in nc.m.functions:
        for bb in fn.blocks:
            out = []
            for inst in bb.instructions:
                si = inst.sync_info
                waits = list(si.on_wait) if si is not None else []
                limit = 1
                if len(waits) > limit:
                    extra, keep = waits[:-limit], waits[-limit:]
                    for i in range(len(extra)):
                        nop = mybir.InstNoOp(
                            name=nc.get_next_instruction_name(), ins=[], outs=[])
                        nop.engine = inst.engine
                        nop.sync_info = bass_rust.SyncInfo(
                            on_wait=[extra[i]], on_update=[])
                        nc.register_instruction(nop)
                        out.append(nop)
                    inst.sync_info = bass_rust.SyncInfo(
                        on_wait=keep, on_update=list(si.on_update))
                out.append(inst)
            bb.instructions[:] = out


def shard_inputs(inputs):
    """Full inputs -> list of 8 per-core in_maps (numpy)."""
    import ml_dtypes
    BF = ml_dtypes.bfloat16
    E8NP = np.dtype(mybir.dt.np(mybir.dt.float8e4))
    e = np.ascontiguousarray(inputs["edge_fts"], dtype=np.float32)
    nf = np.ascontiguousarray(inputs["node_fts"], dtype=np.float32)
    hd = np.ascontiguousarray(inputs["hidden"], dtype=np.float32)
    gfa = np.ascontiguousarray(inputs["graph_fts"], dtype=np.float32)
    adj = np.asarray(inputs["adj_mat"])
    w = {k: np.ascontiguousarray(inputs[k], dtype=np.float32) for k in (
        "m_w", "m_b", "skip_w", "skip_b", "a1_w", "a1_b", "a2_w", "a2_b",
        "ae_w", "ae_b", "ag_w", "ag_b")}
    # ZIN-row weights as [128, (2, n)] bf16 (c-halves side by side)
    def w2(name, ncols):
        return np.ascontiguousarray(
            w[name].reshape(2, 128, ncols).transpose(1, 0, 2)
            .reshape(128, 2 * ncols), dtype=BF)
    # blockdiag bd[(j_lo,e), (j_lo', h)] = ae_w[e, h] * (j_lo == j_lo')
    bdm = np.zeros((8, 16, 8, H), np.float32)
    for jl in range(8):
        bdm[jl, :, jl, :] = w["ae_w"]
    bdm = np.ascontiguousarray(bdm.reshape(128, 64).astype(BF))
    maps = []
    for c in range(NC):
        b, ih = c // 2, c % 2
        i0 = ih * NI
        # For odd cores, rotate the j axis (and z rows) by -512 so that the
        # core's own rows always sit at z columns 0..511. The attention sum
        # over j is permutation-invariant, so rolling e/adj/z consistently
        # leaves the output unchanged.
        ej = e[b, i0:i0 + NI]
        aj = adj[b, i0:i0 + NI, :]
        nfb, hdb = nf[b], hd[b]
        if ih == 1:
            ej = np.roll(ej, -NI, axis=1)
            aj = np.roll(aj, -NI, axis=1)
            nfb = np.roll(nfb, -NI, axis=0)
            hdb = np.roll(hdb, -NI, axis=0)
        # e_oct[grp][(j_lo,e)=128, (i16, j_hi128)=2048] fp8, grp-contiguous
        # ej: [512 i, 1024 j, 16 e] -> [32 grp, 16 i, 128 j_hi, 8 jl, 16 e]
        eo = ej.reshape(32, 16, 128, 8, 16).transpose(0, 3, 4, 1, 2)
        e_oct = np.ascontiguousarray(eo.reshape(32, 128, 2048).astype(E8NP))
        # adj bias slabs: (adj-1)*1e9, cols (i16, j_lo8) per group,
        # 4 groups (=1/2 block) per row: [8, 128 j_hi, (4 grp, 16 i, 8 jl)]
        ab = (aj.astype(np.float32) - 1.0) * 1e9
        # [512 i, 1024 j] -> [8 gp4, 4 grp, 16 i, 128 j_hi, 8 jl]
        a5 = ab.reshape(8, 4, 16, 128, 8).transpose(0, 3, 1, 2, 4)
        adj_g = np.ascontiguousarray(a5.reshape(8, 128, 512), dtype=BF)
        m = {
            "e_oct": e_oct,
            "adj_g": adj_g,
            "zT": np.ascontiguousarray(
                np.concatenate([nfb.T, hdb.T], axis=1).astype(BF)),
            "bd": bdm,
            "gf": gfa[b].reshape(FG, 1).astype(BF),
            "m_w": w2("m_w", OUT), "m_b": w["m_b"].reshape(1, OUT).astype(BF),
            "skip_w": w2("skip_w", OUT),
            "skip_b": w["skip_b"].reshape(1, OUT).astype(BF),
            "a1_w": w2("a1_w", H), "a1_b": w["a1_b"].reshape(1, H),
            "a2_w": w2("a2_w", H), "a2_b": w["a2_b"].reshape(1, H),
            "ae_b": w["ae_b"].reshape(1, H),
            "ag_w": w["ag_w"].astype(BF),
            "ag_b": w["ag_b"].reshape(1, H),
        }
        maps.append(m)
    return maps


def build(n_iters=1):
    """One program shared by all 8 cores (inputs are pre-rotated so own
    rows always sit at z columns 0..511)."""
    nc = bass.Bass("TRN2", target_bir_lowering=False, debug=False,
                   num_devices=NC)
    build_core_program(nc, n_iters=n_iters)
    split_multi_waits(nc)
    return nc


def kernel(**inputs):
    from concourse.bass_utils import run_bass_kernel_spmd
    maps = shard_inputs(inputs)
    nc = build(n_iters=1)
    res = run_bass_kernel_spmd(nc, maps, list(range(NC))).results
    out = np.zeros((B, N, OUT), np.float32)
    for c in range(NC):
        b, ih = c // 2, c % 2
        out[b, ih * NI:(ih + 1) * NI] = res[c]["ret"]
    return out
